# revision 10
# baseline (speedup 1.0000x reference)
"""Trainium2 Bass kernel for nn_DLP_Loss (retrieval_knn).

loss = cross_entropy(scores, target)
     + (0.5/K) * sum_i sum_{k in 5-NN same-class} mean_d (x_i - x_nbr)^2

Strategy (8 NeuronCores, SPMD):
  * Host: stable-sort rows by class. Queries are data-parallel sharded
    (1024 rows/core). Each core receives only the contiguous key window
    covering the classes its queries belong to (padded to a uniform W so
    the single SPMD program works for all cores).
  * Device: for each 128-query tile, PSUM = 2*x_i.x_j - |x_j|^2
    - BIG*(t_i - t_j)^2 via two chained matmuls (K=128 features, then a
    K=4 "mask + key-norm" matmul; the BIG terms cancel exactly for
    same-class pairs and poison different-class/pad columns). Since
    d2(i,j) = |x_i|^2 - PSUM(i,j), the row maximum is always self
    (d2=0) and the next 5 are the 5 nearest same-class neighbors: one
    DVE Max8 instruction per tile gives them with no gather.
    sum_sel d2 = cnt*|x_i|^2 - sum_sel v with |x_i|^2 = Max8 slot 0.
  * Cross-entropy for the core's rows is computed on-chip (Exp/Ln).
  * Each core writes [sum_pair_d2, sum_ce]; host adds the 8 partials.
"""

import os
import sys
import numpy as np

if "/opt/trn_rl_repo" not in sys.path:
    sys.path.insert(0, "/opt/trn_rl_repo")

import concourse.bass as bass
import concourse.bacc as bacc
import concourse.mybir as mybir
import concourse.tile as tile
from concourse import bass_utils

F32 = mybir.dt.float32
F32R = mybir.dt.float32r
AX = mybir.AxisListType
ALU = mybir.AluOpType
ACTF = mybir.ActivationFunctionType

N_CORES = 8
K = 5
BIG = float(2**30)
PADV = 100.0
USE_F32R = os.environ.get("KNN_F32R", "1") == "1"

# test.py introspection: last BassKernelResults from run_bass_kernel_spmd
LAST_RESULTS = None
_PROGRAM_CACHE = {}


def _maybe_enable_trace_hook():
    """Register the axon NTFF profile hook so BASS_TRACE=1 yields exec_time_ns.

    Harmless no-op if the boot shim is unavailable (fresh grading env)."""
    if not os.environ.get("BASS_TRACE"):
        return
    if "antenv.axon_hooks" in sys.modules:
        return
    try:
        import types

        import trn_agent_boot.trn_boot as trn_boot

        mod = types.ModuleType("antenv.axon_hooks")
        hook = [trn_boot._ntff_profile_via_ctypes("/opt/axon/libaxon_pjrt.so")]
        mod.set_axon_ntff_profile_hook = lambda h: hook.__setitem__(0, h)
        mod.get_axon_ntff_profile_hook = lambda: hook[0]
        sys.modules["antenv.axon_hooks"] = mod
    except Exception:
        pass


def _build_program(W, n_tiles):
    """One SPMD program; per-core data differs only through the input maps."""
    nch = W // 512
    nc = bacc.Bacc("TRN2", target_bir_lowering=False, debug=False,
                   num_devices=N_CORES)

    # FP32R = full-rate PE fp32 path (1 cyc/row vs 4). The walrus verifier
    # requires every producer of an FP32R matmul operand to be FP32R-typed,
    # so the four matmul operands are declared FP32R end-to-end (DRAM+SBUF).
    MMDT = F32R if USE_F32R else F32

    npc = n_tiles * 128
    d_q2t = nc.dram_tensor("q2t", (128, npc), MMDT, kind="ExternalInput")
    d_keys = nc.dram_tensor("keyst", (128, W), MMDT, kind="ExternalInput")
    d_mlhs = nc.dram_tensor("mlhst", (4, npc), MMDT, kind="ExternalInput")
    d_mrhs = nc.dram_tensor("mrhs4", (4, W), MMDT, kind="ExternalInput")
    d_scores = nc.dram_tensor("scoresr", (128, n_tiles * 7), F32,
                              kind="ExternalInput")
    d_tq = nc.dram_tensor("tqr", (128, n_tiles), F32, kind="ExternalInput")
    d_out = nc.dram_tensor("out", (1, 8), F32, kind="ExternalOutput")

    with tile.TileContext(nc) as tc:
        with (
            tc.tile_pool(name="big", bufs=1) as big,
            tc.tile_pool(name="drowp", bufs=2) as drowp,
            tc.tile_pool(name="small", bufs=3) as small,
            tc.tile_pool(name="pmain", bufs=4, space=bass.MemorySpace.PSUM) as pmain,
            tc.tile_pool(name="psmall", bufs=2, space=bass.MemorySpace.PSUM) as psmall,
        ):
            keys_sb = big.tile([128, W], MMDT)
            q2t_sb = big.tile([128, npc], MMDT)
            mlhs_sb = big.tile([4, npc], MMDT)
            mrhs_sb = big.tile([4, W], MMDT)
            scores_sb = big.tile([128, n_tiles * 7], F32)
            tq_sb = big.tile([128, n_tiles], F32)
            acc5 = big.tile([128, n_tiles], F32)
            accce = big.tile([128, n_tiles], F32)
            pack2 = big.tile([128, 2], F32)
            ones128 = big.tile([128, 1], F32)
            ci32 = big.tile([128, 7], mybir.dt.int32)
            iof = big.tile([128, 7], F32)
            outsb = big.tile([1, 8], F32)

            nc.gpsimd.memset(ones128[:], 1.0)
            nc.gpsimd.iota(ci32[:], pattern=[[1, 7]], base=0,
                           channel_multiplier=0)
            nc.vector.tensor_copy(iof[:], ci32[:])

            # loads
            for ch in range(nch):
                sl = slice(ch * 512, (ch + 1) * 512)
                nc.sync.dma_start(keys_sb[:, sl], d_keys.ap()[:, sl])
            nc.sync.dma_start(q2t_sb[:], d_q2t.ap())
            nc.sync.dma_start(mlhs_sb[:], d_mlhs.ap())
            nc.sync.dma_start(mrhs_sb[:], d_mrhs.ap())
            nc.sync.dma_start(scores_sb[:], d_scores.ap())
            nc.sync.dma_start(tq_sb[:], d_tq.ap())

            # main: P[i,j] = -BIG*(t_i-t_j)^2 - |x_j|^2 + 2*x_i.x_j
            for t in range(n_tiles):
                tsl = slice(t * 128, (t + 1) * 128)
                drow = drowp.tile([128, W], F32)
                for ch in range(nch):
                    sl = slice(ch * 512, (ch + 1) * 512)
                    pm = pmain.tile([128, 512], F32)
                    nc.tensor.matmul(pm[:], mlhs_sb[:, tsl], mrhs_sb[:, sl],
                                     start=True, stop=False)
                    nc.tensor.matmul(pm[:], q2t_sb[:, tsl], keys_sb[:, sl],
                                     start=False, stop=True)
                    nc.scalar.copy(drow[:, sl], pm[:])

                o8 = small.tile([128, 8], F32)
                nc.vector.max(o8[:], drow[:])
                # slots 1..5 = 5 nearest same-class neighbors (slot 0 = self)
                mask5 = small.tile([128, 5], F32)
                nc.vector.tensor_scalar(out=mask5[:], in0=o8[:, 1:6],
                                        scalar1=-1.0e5, scalar2=None,
                                        op0=ALU.is_gt)
                cnt = small.tile([128, 1], F32)
                nc.vector.reduce_sum(cnt[:], mask5[:], axis=AX.X)
                mv = small.tile([128, 5], F32)
                smv = small.tile([128, 1], F32)
                nc.vector.tensor_mul(mv[:], o8[:, 1:6], mask5[:])
                nc.vector.reduce_sum(smv[:], mv[:], axis=AX.X)
                c1 = small.tile([128, 1], F32)
                nc.vector.tensor_mul(c1[:], cnt[:], o8[:, 0:1])
                nc.vector.tensor_sub(acc5[:, t:t + 1], c1[:], smv[:])

            # cross-entropy rows: ce = max + ln(sum exp(s - max)) - s[target]
            for t in range(n_tiles):
                s_t = scores_sb[:, t * 7:(t + 1) * 7]
                m = small.tile([128, 1], F32)
                nc.vector.reduce_max(m[:], s_t, axis=AX.X)
                negm = small.tile([128, 1], F32)
                nc.vector.tensor_scalar_mul(negm[:], m[:], -1.0)
                e = small.tile([128, 7], F32)
                se = small.tile([128, 1], F32)
                nc.scalar.activation(e[:], s_t, ACTF.Exp, bias=negm[:],
                                     scale=1.0, accum_out=se[:])
                lnse = small.tile([128, 1], F32)
                nc.scalar.activation(lnse[:], se[:], ACTF.Ln)
                cmask = small.tile([128, 7], F32)
                nc.vector.tensor_scalar(out=cmask[:], in0=iof[:],
                                        scalar1=tq_sb[:, t:t + 1],
                                        scalar2=None, op0=ALU.is_equal)
                junk = small.tile([128, 7], F32)
                st = small.tile([128, 1], F32)
                nc.vector.tensor_mul(junk[:], s_t, cmask[:])
                nc.vector.reduce_sum(st[:], junk[:], axis=AX.X)
                t1 = small.tile([128, 1], F32)
                nc.vector.tensor_add(t1[:], m[:], lnse[:])
                nc.vector.tensor_sub(accce[:, t:t + 1], t1[:], st[:])

            # fold partitions: out = [sum pair_d2, sum ce, 0...]
            nc.vector.reduce_sum(pack2[:, 0:1], acc5[:], axis=AX.X)
            nc.vector.reduce_sum(pack2[:, 1:2], accce[:], axis=AX.X)
            pf = psmall.tile([1, 2], F32)
            nc.tensor.matmul(pf[:], ones128[:], pack2[:],
                             start=True, stop=True)
            nc.gpsimd.memset(outsb[:], 0.0)
            nc.scalar.copy(outsb[0:1, 0:2], pf[:])
            nc.sync.dma_start(d_out.ap(), outsb[:])

    nc.compile()
    return nc


def _prep_inputs(x, sc, tg):
    """Sort by class, build the 8 per-core input maps."""
    n, d = x.shape
    npc = n // N_CORES
    perm = np.argsort(tg, kind="stable")
    xs = np.ascontiguousarray(x[perm])
    ss = np.ascontiguousarray(sc[perm])
    ts = tg[perm]
    xsT = np.ascontiguousarray(xs.T)  # (128, N)

    nclass = int(ts.max()) + 1 if n else 1
    clo = np.searchsorted(ts, np.arange(nclass), "left")
    chi = np.searchsorted(ts, np.arange(nclass), "right")
    row_lo = clo[ts]
    row_hi = chi[ts]

    spans = []
    for c in range(N_CORES):
        r0, r1 = c * npc, (c + 1) * npc - 1
        spans.append((int(row_lo[r0]), int(row_hi[r1])))
    wmax = max(hi - lo for lo, hi in spans)
    W = max(512, -(-wmax // 512) * 512)

    tsf = ts.astype(np.float64)
    k2 = (xs.astype(np.float64) ** 2).sum(1)  # |x_j|^2 per sorted row
    in_maps = []
    for c in range(N_CORES):
        r0 = c * npc
        r1 = r0 + npc
        wlo, whi = spans[c]
        ww = whi - wlo

        keys = np.zeros((128, W), np.float32)
        keys[:, :ww] = xsT[:, wlo:whi]

        # pad cols: t=-1 -> penalty <= -BIG for every query class >= 0
        twin = np.full((W,), -1.0, np.float64)
        twin[:ww] = tsf[wlo:whi]
        mrhs4 = np.zeros((4, W), np.float32)
        mrhs4[0] = 1.0
        mrhs4[1] = twin
        mrhs4[2] = -BIG * twin * twin
        mrhs4[3, :ww] = -k2[wlo:whi]

        tq = tsf[r0:r1]
        mlhs = np.empty((4, npc), np.float32)
        mlhs[0] = -BIG * tq * tq
        mlhs[1] = 2.0 * BIG * tq
        mlhs[2] = 1.0
        mlhs[3] = 1.0

        in_maps.append({
            "q2t": np.ascontiguousarray(2.0 * xsT[:, r0:r1]),
            "keyst": keys,
            "mlhst": mlhs,
            "mrhs4": mrhs4,
            "scoresr": np.ascontiguousarray(
                ss[r0:r1].reshape(-1, 128, 7).transpose(1, 0, 2)
            ).reshape(128, -1),
            "tqr": np.ascontiguousarray(
                tq.reshape(-1, 128).T.astype(np.float32)),
        })
    return in_maps, W, npc // 128


def kernel(input, scores, target):
    global LAST_RESULTS
    _maybe_enable_trace_hook()

    x = np.asarray(input, np.float32)
    sc = np.asarray(scores, np.float32)
    tg = np.asarray(target).astype(np.int64)
    n, d = x.shape

    in_maps, W, n_tiles = _prep_inputs(x, sc, tg)

    key = (W, n_tiles)
    if key not in _PROGRAM_CACHE:
        _PROGRAM_CACHE[key] = _build_program(W, n_tiles)
    nc = _PROGRAM_CACHE[key]

    res = bass_utils.run_bass_kernel_spmd(
        nc, in_maps, core_ids=list(range(N_CORES)))
    LAST_RESULTS = res

    pair_d2 = 0.0
    ce_sum = 0.0
    for r in res.results:
        o = np.asarray(r["out"], np.float64).reshape(-1)
        pair_d2 += o[0]
        ce_sum += o[1]

    loss = ce_sum / n + pair_d2 * 0.5 / (K * d)
    return np.float32(loss)
